# revision 6
# baseline (speedup 1.0000x reference)
"""Causal dilated 1D conv (KW=4, dilation=8) as shifted matmuls on 8 TRN2 cores.

out[b,o,t] = sum_{k,c} W[o, c*4+k] * x[b, c, t + k*8 - 24]

Sharding: data-parallel over batch (16 batches -> 2 per core). Each core runs
an identical program: all weights stationary in SBUF, x streamed in 512-wide
time blocks (+24 halo), 16 accumulating matmuls (4 c-chunks x 4 taps) per
(out-chunk, time-block) PSUM group, PSUM copied back via DVE and DMA'd out.

Matmuls run in bfloat16 (fp32 PSUM accumulate): 1 cycle/row streaming, and
unlike fp32/f32r the compiler-automatic Fast Weight Load path is enabled, so
the per-matmul LDWEIGHTS (97ns) hides under the previous matmul's 512-row
stream; measured steady-state cadence 216ns/MM = 512rows/2.4GHz + ~3ns NX
issue overhead, which is the HW floor. bf16 quantization of x and W gives
~2.3e-3 relative error over the K=2048 contraction (gate 2e-2; fp8
DoubleRow measured 4e-2 on this data -> unusable).

Edge optimizations (the steady state has no gaps):
- Warm-up MMs on a zeroed scratch tile keep the PE busy from the preamble
  barrier on, so the HAM activity monitor unthrottles the PE clock
  (1.2->2.4GHz after 3.4us of sustained activity) while the bootstrap DMAs
  are still in flight, and the first real MM runs at full clock.
- DMA is split across both hardware DGE queues (sync/SP + scalar/Act):
  bootstrap weight tiles alternate queues so first-group inputs land ~2x
  sooner; steady-state output writes ride the scalar queue, inputs the sync
  queue.
- The last PSUM group's drain is split in half across DVE+Act copies and
  both DMA queues to shorten the serial tail.
"""

import ml_dtypes
import numpy as np

B = 16
C_IN = 512
C_OUT = 512
T = 8192
KW = 4
DIL = 8
PAD = (KW - 1) * DIL  # 24

N_CORES = 8
B_PER = B // N_CORES  # 2
P = 128
TBLK = 512
NT = T // TBLK        # 16
NCC = C_IN // P       # 4
NOC = C_OUT // P      # 4
N_WARM = 28

_cache = {}


def _build():
    import concourse.tile as tile
    from concourse import bacc, mybir

    nc = bacc.Bacc("TRN2", target_bir_lowering=False, debug=False,
                   num_devices=N_CORES)
    x = nc.dram_tensor("x", [B_PER, C_IN, T + PAD], mybir.dt.bfloat16,
                       kind="ExternalInput").ap()
    # weights pre-arranged on host as [cc, tap, c=128, o=512]
    wt = nc.dram_tensor("wt", [NCC, KW, P, C_OUT], mybir.dt.bfloat16,
                        kind="ExternalInput").ap()
    out = nc.dram_tensor("out", [B_PER, C_OUT, T], mybir.dt.float32,
                         kind="ExternalOutput").ap()
    f32 = mybir.dt.float32
    bf16 = mybir.dt.bfloat16

    with tile.TileContext(nc) as tc:
        with tc.tile_pool(name="wpool", bufs=1) as wpool, \
             tc.tile_pool(name="xpool", bufs=8) as xpool, \
             tc.tile_pool(name="opool", bufs=8) as opool, \
             tc.tile_pool(name="pspool", bufs=8, space="PSUM") as pspool:

            # PE warm-up: zero a scratch tile and issue dependency-free MMs
            # so the PE is busy (and the HAM clock warm) during the DMA
            # bootstrap below.
            warm = wpool.tile([P, P + TBLK], bf16, name="warm", tag="warm")
            nc.vector.memset(warm[:], 0)
            wps = pspool.tile([P, TBLK], f32, name="ps", tag="ps")
            for _ in range(N_WARM):
                nc.tensor.matmul(wps[:], warm[:, :P], warm[:, P:],
                                 start=True, stop=True)

            def load_xt(b, tb):
                xts = []
                for cc in range(NCC):
                    xt = xpool.tile([P, TBLK + PAD], bf16,
                                    name=f"xt{cc}", tag=f"xt{cc}")
                    nc.sync.dma_start(
                        xt[:],
                        x[b, cc * P:(cc + 1) * P,
                          tb * TBLK: tb * TBLK + TBLK + PAD])
                    xts.append(xt)
                return xts

            # Bootstrap loads: x tiles stream on the sync queue while the 16
            # weight tiles alternate between the two HW DGE queues, ordered
            # so the first-group inputs (xt0, w_0_*) lead both wire streams.
            first_xts = load_xt(0, 0)
            wtiles = [[None] * KW for _ in range(NCC)]
            for cc in range(NCC):
                for k in range(KW):
                    wtile = wpool.tile([P, C_OUT], bf16, name=f"w_{cc}_{k}",
                                       tag=f"w_{cc}_{k}")
                    eng = nc.scalar if cc < 2 else nc.sync
                    eng.dma_start(wtile[:], wt[cc, k])
                    wtiles[cc][k] = wtile

            n_acc = NCC * KW
            cks = [(cc, k) for cc in range(NCC) for k in range(KW)]

            # Bootstrap block: emit MMs in weight-DMA-arrival order, fanning
            # each arriving weight across the 4 oc PSUM banks, so the in-order
            # PE stream is never head-of-line blocked on a later weight tile.
            pss0 = [pspool.tile([P, TBLK], f32, name="ps", tag="ps")
                    for _ in range(NOC)]
            for ci, (cc, k) in enumerate(cks):
                for oc in range(NOC):
                    nc.tensor.matmul(
                        pss0[oc][:],
                        wtiles[cc][k][:, oc * P:(oc + 1) * P],
                        first_xts[cc][:, k * DIL: k * DIL + TBLK],
                        start=(ci == 0),
                        stop=(ci == n_acc - 1),
                    )
            for oc in range(NOC):
                ot = opool.tile([P, TBLK], f32, name="ot", tag="ot")
                nc.vector.tensor_copy(ot[:], pss0[oc][:])
                nc.scalar.dma_start(out[0, oc * P:(oc + 1) * P, 0:TBLK],
                                    ot[:])

            H = TBLK // 2
            for b in range(B_PER):
                for tb in range(NT):
                    if b == 0 and tb == 0:
                        continue
                    last = (b == B_PER - 1 and tb == NT - 1)
                    xts = load_xt(b, tb)
                    for oc in range(NOC):
                        ps = pspool.tile([P, TBLK], f32, name="ps", tag="ps")
                        for ci, (cc, k) in enumerate(cks):
                            nc.tensor.matmul(
                                ps[:],
                                wtiles[cc][k][:, oc * P:(oc + 1) * P],
                                xts[cc][:, k * DIL: k * DIL + TBLK],
                                start=(ci == 0),
                                stop=(ci == n_acc - 1),
                            )
                        ot = opool.tile([P, TBLK], f32, name="ot", tag="ot")
                        orow = out[b, oc * P:(oc + 1) * P,
                                   tb * TBLK:(tb + 1) * TBLK]
                        if last and oc == NOC - 1:
                            # Drain the final group through both copy engines
                            # and both DMA queues to halve the serial tail.
                            nc.vector.tensor_copy(ot[:, :H], ps[:, :H])
                            nc.sync.dma_start(orow[:, :H], ot[:, :H])
                            nc.scalar.copy(ot[:, H:], ps[:, H:])
                            nc.scalar.dma_start(orow[:, H:], ot[:, H:])
                        else:
                            nc.vector.tensor_copy(ot[:], ps[:])
                            nc.scalar.dma_start(orow, ot[:])

    nc.compile()
    return nc


def _get_nc():
    if "nc" not in _cache:
        _cache["nc"] = _build()
    return _cache["nc"]


def _make_in_maps(x, W):
    xb = np.ascontiguousarray(x, dtype=np.float32).astype(ml_dtypes.bfloat16)
    xpad = np.pad(xb, ((0, 0), (0, 0), (PAD, 0)))
    w = np.ascontiguousarray(W, dtype=np.float32).reshape(C_OUT, C_IN, KW)
    # wt[cc, k, c, o] = W[o, (cc*128+c)*KW + k]
    wt = np.transpose(w.reshape(C_OUT, NCC, P, KW),
                      (1, 3, 2, 0)).astype(ml_dtypes.bfloat16).copy()
    return [{"x": np.ascontiguousarray(xpad[i * B_PER:(i + 1) * B_PER]),
             "wt": wt} for i in range(N_CORES)]


def kernel(x, W):
    from concourse.bass_utils import run_bass_kernel_spmd

    nc = _get_nc()
    in_maps = _make_in_maps(x, W)
    res = run_bass_kernel_spmd(nc, in_maps, list(range(N_CORES)))
    return np.concatenate([r["out"] for r in res.results], axis=0)


# revision 10
# speedup vs baseline: 1.0057x; 1.0057x over previous
"""Causal dilated 1D conv (KW=4, dilation=8) as shifted matmuls on 8 TRN2 cores.

out[b,o,t] = sum_{k,c} W[o, c*4+k] * x[b, c, t + k*8 - 24]

Sharding: data-parallel over batch (16 batches -> 2 per core). Each core runs
an identical program: all weights stationary in SBUF, x streamed in 512-wide
time blocks (+24 halo), 16 accumulating matmuls (4 c-chunks x 4 taps) per
(out-chunk, time-block) PSUM group, PSUM copied back via DVE and DMA'd out.

Matmuls run in bfloat16 (fp32 PSUM accumulate): 1 cycle/row streaming, and
unlike fp32/f32r the compiler-automatic Fast Weight Load path is enabled, so
the per-matmul LDWEIGHTS (97ns) hides under the previous matmul's 512-row
stream; measured steady-state cadence 216ns/MM = 512rows/2.4GHz + ~3ns NX
issue overhead, which is the HW floor. bf16 quantization of x and W gives
~2.3e-3 relative error over the K=2048 contraction (gate 2e-2; fp8
DoubleRow measured 4e-2 on this data -> unusable).

Edge optimizations (the steady state has no gaps):
- Warm-up MMs on a zeroed scratch tile keep the PE busy from the preamble
  barrier on, so the HAM activity monitor unthrottles the PE clock
  (1.2->2.4GHz after 3.4us of sustained activity) while the bootstrap DMAs
  are still in flight, and the first real MM runs at full clock.
- DMA is split across both hardware DGE queues (sync/SP + scalar/Act):
  bootstrap weight tiles alternate queues so first-group inputs land ~2x
  sooner; steady-state output writes ride the scalar queue, inputs the sync
  queue.
- The last PSUM group's drain is split in half across DVE+Act copies and
  both DMA queues to shorten the serial tail.
"""

import ml_dtypes
import numpy as np

B = 16
C_IN = 512
C_OUT = 512
T = 8192
KW = 4
DIL = 8
PAD = (KW - 1) * DIL  # 24

N_CORES = 8
B_PER = B // N_CORES  # 2
P = 128
TBLK = 512
NT = T // TBLK        # 16
NCC = C_IN // P       # 4
NOC = C_OUT // P      # 4
N_WARM = 8

_cache = {}


def _build():
    import concourse.tile as tile
    from concourse import bacc, mybir

    nc = bacc.Bacc("TRN2", target_bir_lowering=False, debug=False,
                   num_devices=N_CORES)
    x = nc.dram_tensor("x", [B_PER, C_IN, T + PAD], mybir.dt.bfloat16,
                       kind="ExternalInput").ap()
    # weights pre-arranged on host as [cc, tap, c=128, o=512]
    wt = nc.dram_tensor("wt", [NCC, KW, P, C_OUT], mybir.dt.bfloat16,
                        kind="ExternalInput").ap()
    out = nc.dram_tensor("out", [B_PER, C_OUT, T], mybir.dt.float32,
                         kind="ExternalOutput").ap()
    f32 = mybir.dt.float32
    bf16 = mybir.dt.bfloat16

    with tile.TileContext(nc) as tc:
        with tc.tile_pool(name="wpool", bufs=1) as wpool, \
             tc.tile_pool(name="xpool", bufs=8) as xpool, \
             tc.tile_pool(name="opool", bufs=8) as opool, \
             tc.tile_pool(name="pspool", bufs=8, space="PSUM") as pspool:

            # PE warm-up: zero a scratch tile and issue dependency-free MMs
            # so the PE is busy (and the HAM clock warm) during the DMA
            # bootstrap below.
            warm = wpool.tile([P, P + TBLK], bf16, name="warm", tag="warm")
            nc.vector.memset(warm[:], 0)
            wps = pspool.tile([P, TBLK], f32, name="ps", tag="ps", bufs=4)
            for _ in range(N_WARM):
                nc.tensor.matmul(wps[:], warm[:, :P], warm[:, P:],
                                 start=True, stop=True)

            def load_xt(b, tb):
                xts = []
                for cc in range(NCC):
                    xt = xpool.tile([P, TBLK + PAD], bf16,
                                    name=f"xt{cc}", tag=f"xt{cc}")
                    nc.sync.dma_start(
                        xt[:],
                        x[b, cc * P:(cc + 1) * P,
                          tb * TBLK: tb * TBLK + TBLK + PAD])
                    xts.append(xt)
                return xts

            # Bootstrap loads: x tiles stream on the sync queue while the 16
            # weight tiles alternate between the two HW DGE queues, ordered
            # so the first-group inputs (xt0, w_0_*) lead both wire streams.
            first_xts = load_xt(0, 0)
            wtiles = [[None] * KW for _ in range(NCC)]
            for cc in range(NCC):
                for k in range(KW):
                    wtile = wpool.tile([P, C_OUT], bf16, name=f"w_{cc}_{k}",
                                       tag=f"w_{cc}_{k}")
                    eng = nc.scalar if cc < 2 else nc.sync
                    eng.dma_start(wtile[:], wt[cc, k])
                    wtiles[cc][k] = wtile

            n_acc = NCC * KW
            cks = [(cc, k) for cc in range(NCC) for k in range(KW)]

            # Bootstrap block: emit MMs in weight-DMA-arrival order, fanning
            # each arriving weight across the 4 oc PSUM banks, so the in-order
            # PE stream is never head-of-line blocked on a later weight tile.
            pss0 = [pspool.tile([P, TBLK], f32, name="ps", tag="ps",
                                bufs=4)
                    for _ in range(NOC)]
            for ci, (cc, k) in enumerate(cks):
                for oc in range(NOC):
                    nc.tensor.matmul(
                        pss0[oc][:],
                        wtiles[cc][k][:, oc * P:(oc + 1) * P],
                        first_xts[cc][:, k * DIL: k * DIL + TBLK],
                        start=(ci == 0),
                        stop=(ci == n_acc - 1),
                    )
            for oc in range(NOC):
                ot = opool.tile([P, TBLK], f32, name="ot", tag="ot")
                nc.vector.tensor_copy(ot[:], pss0[oc][:])
                nc.scalar.dma_start(out[0, oc * P:(oc + 1) * P, 0:TBLK],
                                    ot[:])

            H = TBLK // 2
            for b in range(B_PER):
                for tb in range(NT):
                    if b == 0 and tb == 0:
                        continue
                    last = (b == B_PER - 1 and tb == NT - 1)
                    xts = load_xt(b, tb)
                    for oc in range(NOC):
                        if not last:
                            ps = pspool.tile([P, TBLK], f32, name="ps",
                                             tag="ps", bufs=4)
                            for ci, (cc, k) in enumerate(cks):
                                nc.tensor.matmul(
                                    ps[:],
                                    wtiles[cc][k][:, oc * P:(oc + 1) * P],
                                    xts[cc][:, k * DIL: k * DIL + TBLK],
                                    start=(ci == 0),
                                    stop=(ci == n_acc - 1),
                                )
                            ot = opool.tile([P, TBLK], f32,
                                            name="ot", tag="ot")
                            nc.vector.tensor_copy(ot[:], ps[:])
                            nc.scalar.dma_start(
                                out[b, oc * P:(oc + 1) * P,
                                    tb * TBLK:(tb + 1) * TBLK], ot[:])
                            continue
                        # Last time block: two 256-wide half-blocks so the
                        # final PSUM drain (copy + out-DMA) is half as deep;
                        # the very last drain additionally splits across both
                        # copy engines and both DMA queues.
                        for h in range(2):
                            ps = pspool.tile([P, H], f32, name="psh",
                                             tag="psh", bufs=4)
                            for ci, (cc, k) in enumerate(cks):
                                nc.tensor.matmul(
                                    ps[:],
                                    wtiles[cc][k][:, oc * P:(oc + 1) * P],
                                    xts[cc][:, h * H + k * DIL:
                                            h * H + k * DIL + H],
                                    start=(ci == 0),
                                    stop=(ci == n_acc - 1),
                                )
                            ot = opool.tile([P, H], f32, name="oth",
                                            tag="oth")
                            orow = out[b, oc * P:(oc + 1) * P,
                                       tb * TBLK + h * H:
                                       tb * TBLK + (h + 1) * H]
                            if oc == NOC - 1 and h == 1:
                                Q = H // 2
                                nc.vector.tensor_copy(ot[:, :Q], ps[:, :Q])
                                nc.sync.dma_start(orow[:, :Q], ot[:, :Q])
                                nc.scalar.copy(ot[:, Q:], ps[:, Q:])
                                nc.scalar.dma_start(orow[:, Q:], ot[:, Q:])
                            else:
                                nc.vector.tensor_copy(ot[:], ps[:])
                                nc.scalar.dma_start(orow, ot[:])

    nc.compile()
    return nc


def _get_nc():
    if "nc" not in _cache:
        _cache["nc"] = _build()
    return _cache["nc"]


def _make_in_maps(x, W):
    xb = np.ascontiguousarray(x, dtype=np.float32).astype(ml_dtypes.bfloat16)
    xpad = np.pad(xb, ((0, 0), (0, 0), (PAD, 0)))
    w = np.ascontiguousarray(W, dtype=np.float32).reshape(C_OUT, C_IN, KW)
    # wt[cc, k, c, o] = W[o, (cc*128+c)*KW + k]
    wt = np.transpose(w.reshape(C_OUT, NCC, P, KW),
                      (1, 3, 2, 0)).astype(ml_dtypes.bfloat16).copy()
    return [{"x": np.ascontiguousarray(xpad[i * B_PER:(i + 1) * B_PER]),
             "wt": wt} for i in range(N_CORES)]


def kernel(x, W):
    from concourse.bass_utils import run_bass_kernel_spmd

    nc = _get_nc()
    in_maps = _make_in_maps(x, W)
    res = run_bass_kernel_spmd(nc, in_maps, list(range(N_CORES)))
    return np.concatenate([r["out"] for r in res.results], axis=0)


# revision 11
# speedup vs baseline: 1.0069x; 1.0012x over previous
"""Causal dilated 1D conv (KW=4, dilation=8) as shifted matmuls on 8 TRN2 cores.

out[b,o,t] = sum_{k,c} W[o, c*4+k] * x[b, c, t + k*8 - 24]

Sharding: data-parallel over batch (16 batches -> 2 per core). Each core runs
an identical program: all weights stationary in SBUF, x streamed in 512-wide
time blocks (+24 halo), 16 accumulating matmuls (4 c-chunks x 4 taps) per
(out-chunk, time-block) PSUM group, PSUM copied back via DVE and DMA'd out.

Matmuls run in bfloat16 (fp32 PSUM accumulate): 1 cycle/row streaming, and
unlike fp32/f32r the compiler-automatic Fast Weight Load path is enabled, so
the per-matmul LDWEIGHTS (97ns) hides under the previous matmul's 512-row
stream; measured steady-state cadence 216ns/MM = 512rows/2.4GHz + ~3ns NX
issue overhead, which is the HW floor (443us for 2048 MMs/core). bf16
quantization of x and W gives ~2.3e-3 relative error over the K=2048
contraction (gate 2e-2; fp8 DoubleRow measured 4e-2 on this data).

Edge optimizations (the steady state has no gaps):
- Warm-up MMs on a zeroed scratch tile keep the PE busy from the preamble
  barrier on, so the HAM activity monitor unthrottles the PE clock
  (1.2->2.4GHz after ~3.4us of sustained activity) while the bootstrap DMAs
  are still in flight, and the first real MMs run at full clock.
- DMA is split across both hardware DGE queues (sync/SP + scalar/Act):
  bootstrap weight tiles split across queues so first-group inputs land ~2x
  sooner; steady-state output writes ride the scalar queue, inputs the sync
  queue.
- Tile buffers are merged/minimized (one x tile per time block, one weight
  tile per c-chunk, small pool rings): the NEFF epilogue resets every
  allocated semaphore serially per engine at ~45-90ns each, so each tile
  buffer (~2 semaphores) has a direct exec-time cost. Chunked DMAs into
  disjoint slices keep bootstrap arrival granularity via subtile deps.
"""

import ml_dtypes
import numpy as np

B = 16
C_IN = 512
C_OUT = 512
T = 8192
KW = 4
DIL = 8
PAD = (KW - 1) * DIL  # 24

N_CORES = 8
B_PER = B // N_CORES  # 2
P = 128
TBLK = 512
XW = TBLK + PAD       # 536
NT = T // TBLK        # 16
NCC = C_IN // P       # 4
NOC = C_OUT // P      # 4
N_WARM = 6

_cache = {}


def _build():
    import concourse.tile as tile
    from concourse import bacc, mybir

    nc = bacc.Bacc("TRN2", target_bir_lowering=False, debug=False,
                   num_devices=N_CORES)
    x = nc.dram_tensor("x", [B_PER, C_IN, T + PAD], mybir.dt.bfloat16,
                       kind="ExternalInput").ap()
    # weights pre-arranged on host as [cc, c=128, tap, o=512]
    wt = nc.dram_tensor("wt", [NCC, P, KW, C_OUT], mybir.dt.bfloat16,
                        kind="ExternalInput").ap()
    out = nc.dram_tensor("out", [B_PER, C_OUT, T], mybir.dt.float32,
                         kind="ExternalOutput").ap()
    f32 = mybir.dt.float32
    bf16 = mybir.dt.bfloat16

    with tile.TileContext(nc) as tc:
        with tc.tile_pool(name="wpool", bufs=1) as wpool, \
             tc.tile_pool(name="xpool", bufs=4) as xpool, \
             tc.tile_pool(name="opool", bufs=4) as opool, \
             tc.tile_pool(name="pspool", bufs=4, space="PSUM") as pspool:

            # PE warm-up: zero a scratch tile and issue dependency-free MMs
            # so the PE is busy (and the HAM clock warm) during the DMA
            # bootstrap below.
            warm = wpool.tile([P, P + TBLK], bf16, name="warm", tag="warm")
            nc.vector.memset(warm[:], 0)
            wps = pspool.tile([P, TBLK], f32, name="ps", tag="ps")
            for _ in range(N_WARM):
                nc.tensor.matmul(wps[:], warm[:, :P], warm[:, P:],
                                 start=True, stop=True)

            def load_xt(b, tb):
                # One tile holds all 4 c-chunks side by side; 4 chunk DMAs
                # keep per-chunk arrival granularity via subtile deps.
                xt = xpool.tile([P, NCC * XW], bf16, name="xt", tag="xt")
                for cc in range(NCC):
                    nc.sync.dma_start(
                        xt[:, cc * XW:(cc + 1) * XW],
                        x[b, cc * P:(cc + 1) * P,
                          tb * TBLK: tb * TBLK + XW])
                return xt

            def rhs(xt, cc, k, w=TBLK):
                o = cc * XW + k * DIL
                return xt[:, o: o + w]

            def lhsT(wt_cc, k, oc):
                o = k * C_OUT + oc * P
                return wt_cc[:, o: o + P]

            # Bootstrap loads: x chunks stream on the sync queue; weight
            # tiles (one per c-chunk, 4 per-tap chunk DMAs each) split
            # across the two HW DGE queues.
            xt0 = load_xt(0, 0)
            wtiles = []
            for cc in range(NCC):
                wtile = wpool.tile([P, KW * C_OUT], bf16, name=f"w{cc}",
                                   tag=f"w{cc}")
                eng = nc.scalar if cc < 2 else nc.sync
                for k in range(KW):
                    eng.dma_start(wtile[:, k * C_OUT:(k + 1) * C_OUT],
                                  wt[cc, :, k, :])
                wtiles.append(wtile)

            n_acc = NCC * KW
            cks = [(cc, k) for cc in range(NCC) for k in range(KW)]

            # Bootstrap block: emit MMs in weight-DMA-arrival order, fanning
            # each arriving weight chunk across the 4 oc PSUM banks, so the
            # in-order PE stream is never head-of-line blocked on a later
            # weight chunk.
            pss0 = [pspool.tile([P, TBLK], f32, name="ps", tag="ps")
                    for _ in range(NOC)]
            for ci, (cc, k) in enumerate(cks):
                for oc in range(NOC):
                    nc.tensor.matmul(
                        pss0[oc][:],
                        lhsT(wtiles[cc], k, oc),
                        rhs(xt0, cc, k),
                        start=(ci == 0),
                        stop=(ci == n_acc - 1),
                    )
            for oc in range(NOC):
                ot = opool.tile([P, TBLK], f32, name="ot", tag="ot")
                nc.vector.tensor_copy(ot[:], pss0[oc][:])
                nc.scalar.dma_start(out[0, oc * P:(oc + 1) * P, 0:TBLK],
                                    ot[:])

            for b in range(B_PER):
                for tb in range(NT):
                    if b == 0 and tb == 0:
                        continue
                    xt = load_xt(b, tb)
                    for oc in range(NOC):
                        ps = pspool.tile([P, TBLK], f32, name="ps", tag="ps")
                        for ci, (cc, k) in enumerate(cks):
                            nc.tensor.matmul(
                                ps[:],
                                lhsT(wtiles[cc], k, oc),
                                rhs(xt, cc, k),
                                start=(ci == 0),
                                stop=(ci == n_acc - 1),
                            )
                        ot = opool.tile([P, TBLK], f32, name="ot", tag="ot")
                        nc.vector.tensor_copy(ot[:], ps[:])
                        nc.scalar.dma_start(
                            out[b, oc * P:(oc + 1) * P,
                                tb * TBLK:(tb + 1) * TBLK], ot[:])

    nc.compile()
    return nc


def _get_nc():
    if "nc" not in _cache:
        _cache["nc"] = _build()
    return _cache["nc"]


def _make_in_maps(x, W):
    xb = np.ascontiguousarray(x, dtype=np.float32).astype(ml_dtypes.bfloat16)
    xpad = np.pad(xb, ((0, 0), (0, 0), (PAD, 0)))
    w = np.ascontiguousarray(W, dtype=np.float32).reshape(C_OUT, C_IN, KW)
    # wt[cc, c, k, o] = W[o, (cc*128+c)*KW + k]
    wt = np.transpose(w.reshape(C_OUT, NCC, P, KW),
                      (1, 2, 3, 0)).astype(ml_dtypes.bfloat16).copy()
    return [{"x": np.ascontiguousarray(xpad[i * B_PER:(i + 1) * B_PER]),
             "wt": wt} for i in range(N_CORES)]


def kernel(x, W):
    from concourse.bass_utils import run_bass_kernel_spmd

    nc = _get_nc()
    in_maps = _make_in_maps(x, W)
    res = run_bass_kernel_spmd(nc, in_maps, list(range(N_CORES)))
    return np.concatenate([r["out"] for r in res.results], axis=0)
